# revision 2
# baseline (speedup 1.0000x reference)
"""Trainium2 Bass kernel for the channel-gate MLP problem.

Computes, per batch element b:
    h      = semantic[b] @ W1.T + b1        (256 -> 256)
    h      = leaky_relu(h, 0.1)
    logits = h @ W2.T + b2                  (256 -> 256)
    w      = softmax(logits)
    out[b] = x[b] * (1 + w[:, None, None])

Sharding: pure data parallel over the batch axis (B=8 -> 8 NeuronCores).
Each core gets x[b] as [C=256, H*W=65536] f32 plus replicated (tiny)
MLP weights.  The MLP runs on PE/ACT/DVE and fully overlaps with the
streaming x loads; the bulk of the kernel is a memory-bound
load -> per-partition tensor_scalar multiply -> store pipeline.

Weights are passed pre-transposed (W.T) so both matvecs map directly
onto the tensor engine's lhsT layout with no on-device transpose.
"""

import numpy as np

import concourse.bacc as bacc
import concourse.bass as bass
import concourse.mybir as mybir
import concourse.tile as tile
from concourse.bass_utils import run_bass_kernel_spmd

B = 8
C = 256
HW = 256 * 256  # per-channel spatial size (flattened)
P = 128  # SBUF partitions

F32 = mybir.dt.float32
AX = mybir.AxisListType
AF = mybir.ActivationFunctionType


def build_nc(hw: int = HW, fd: int = 8192, bufs: int = 5):
    """Build the per-core Bass program (identical on all 8 cores)."""
    assert hw % fd == 0
    n_chunks = hw // fd

    nc = bacc.Bacc("TRN2", target_bir_lowering=False, debug=False)

    x_d = nc.declare_dram_parameter("x", [C, hw], F32, isOutput=False)
    sem_d = nc.declare_dram_parameter("semantic", [C, 1], F32, isOutput=False)
    w1t_d = nc.declare_dram_parameter("w1t", [C, C], F32, isOutput=False)
    b1_d = nc.declare_dram_parameter("b1", [C, 1], F32, isOutput=False)
    w2t_d = nc.declare_dram_parameter("w2t", [C, C], F32, isOutput=False)
    b2_d = nc.declare_dram_parameter("b2", [1, C], F32, isOutput=False)
    out_d = nc.declare_dram_parameter("out", [C, hw], F32, isOutput=True)

    with tile.TileContext(nc) as tc:
        with (
            tc.tile_pool(name="const", bufs=1) as cpool,
            tc.tile_pool(name="psum", bufs=1, space="PSUM") as ppool,
            tc.tile_pool(name="big", bufs=bufs) as big,
        ):
            # ---- MLP parameter loads (SWDGE queue: stays off the big
            # HWDGE load/store rings so the MLP never queues behind an
            # 4 MiB x-tile transfer).
            w1t_lo = cpool.tile([P, C], F32, tag="w1t_lo")
            w1t_hi = cpool.tile([P, C], F32, tag="w1t_hi")
            w2t_lo = cpool.tile([P, C], F32, tag="w2t_lo")
            w2t_hi = cpool.tile([P, C], F32, tag="w2t_hi")
            sem_lo = cpool.tile([P, 1], F32, tag="sem_lo")
            sem_hi = cpool.tile([P, 1], F32, tag="sem_hi")
            b1_lo = cpool.tile([P, 1], F32, tag="b1_lo")
            b1_hi = cpool.tile([P, 1], F32, tag="b1_hi")
            b2_row = cpool.tile([1, C], F32, tag="b2_row")
            nc.gpsimd.dma_start(out=w1t_lo[:], in_=w1t_d[0:P, :])
            nc.gpsimd.dma_start(out=w1t_hi[:], in_=w1t_d[P:C, :])
            nc.gpsimd.dma_start(out=w2t_lo[:], in_=w2t_d[0:P, :])
            nc.gpsimd.dma_start(out=w2t_hi[:], in_=w2t_d[P:C, :])
            nc.gpsimd.dma_start(out=sem_lo[:], in_=sem_d[0:P, :])
            nc.gpsimd.dma_start(out=sem_hi[:], in_=sem_d[P:C, :])
            nc.gpsimd.dma_start(out=b1_lo[:], in_=b1_d[0:P, :])
            nc.gpsimd.dma_start(out=b1_hi[:], in_=b1_d[P:C, :])
            nc.gpsimd.dma_start(out=b2_row[:], in_=b2_d[:])

            # ---- layer 1: h = W1 @ semantic  (h[m] = sum_k W1T[k,m] s[k])
            psum_ha = ppool.tile([P, 1], F32, tag="psum_ha")
            psum_hb = ppool.tile([P, 1], F32, tag="psum_hb")
            nc.tensor.matmul(psum_ha[:], w1t_lo[:, 0:P], sem_lo[:], start=True, stop=False)
            nc.tensor.matmul(psum_ha[:], w1t_hi[:, 0:P], sem_hi[:], start=False, stop=True)
            nc.tensor.matmul(psum_hb[:], w1t_lo[:, P:C], sem_lo[:], start=True, stop=False)
            nc.tensor.matmul(psum_hb[:], w1t_hi[:, P:C], sem_hi[:], start=False, stop=True)

            # h = leaky_relu(h + b1) = max(t, 0.1*t) with t = h + b1, PSUM -> SBUF
            h_a = cpool.tile([P, 1], F32, tag="h_a")
            h_b = cpool.tile([P, 1], F32, tag="h_b")
            t_a = cpool.tile([P, 1], F32, tag="t_a")
            t_b = cpool.tile([P, 1], F32, tag="t_b")
            nc.vector.tensor_add(t_a[:], psum_ha[:], b1_lo[:])
            nc.vector.tensor_add(t_b[:], psum_hb[:], b1_hi[:])
            nc.vector.tensor_scalar_mul(h_a[:], t_a[:], 0.1)
            nc.vector.tensor_scalar_mul(h_b[:], t_b[:], 0.1)
            nc.vector.tensor_max(h_a[:], h_a[:], t_a[:])
            nc.vector.tensor_max(h_b[:], h_b[:], t_b[:])

            # ---- layer 2: logits[n] = sum_j h[j] W2T[j,n], as a [1, 256] row
            psum_l = ppool.tile([1, C], F32, tag="psum_l")
            nc.tensor.matmul(psum_l[:], h_a[:], w2t_lo[:], start=True, stop=False)
            nc.tensor.matmul(psum_l[:], h_b[:], w2t_hi[:], start=False, stop=True)

            # ---- softmax over the 256 logits (all in the free dim)
            l_row = cpool.tile([1, C], F32, tag="l_row")
            nc.vector.tensor_add(l_row[:], psum_l[:], b2_row[:])
            mx = cpool.tile([1, 1], F32, tag="mx")
            nc.vector.tensor_reduce(mx[:], l_row[:], axis=AX.X, op=mybir.AluOpType.max)
            neg_mx = cpool.tile([1, 1], F32, tag="neg_mx")
            nc.vector.tensor_scalar_mul(neg_mx[:], mx[:], -1.0)
            e_row = cpool.tile([1, C], F32, tag="e_row")
            e_sum = cpool.tile([1, 1], F32, tag="e_sum")
            nc.scalar.activation(
                e_row[:], l_row[:], AF.Exp, bias=neg_mx[:], scale=1.0, accum_out=e_sum[:]
            )
            r_sum = cpool.tile([1, 1], F32, tag="r_sum")
            nc.vector.reciprocal(r_sum[:], e_sum[:])
            # sc = 1 + softmax = e * (1/sum) + 1
            sc_row = cpool.tile([1, C], F32, tag="sc_row")
            nc.vector.tensor_scalar(
                sc_row[:], e_row[:], r_sum[:], 1.0,
                op0=mybir.AluOpType.mult, op1=mybir.AluOpType.add,
            )

            # ---- move the 256 scales from the free dim onto partitions
            sc_a = cpool.tile([P, 1], F32, tag="sc_a")
            sc_b = cpool.tile([P, 1], F32, tag="sc_b")
            nc.gpsimd.dma_start(out=sc_a[:], in_=sc_row[0:1, 0:P])
            nc.gpsimd.dma_start(out=sc_b[:], in_=sc_row[0:1, P:C])
            scs = [sc_a, sc_b]

            # ---- streaming scale: out = x * sc   (memory-bound main loop)
            for rg in range(2):
                rows = slice(rg * P, (rg + 1) * P)
                for j in range(n_chunks):
                    cols = slice(j * fd, (j + 1) * fd)
                    t = big.tile([P, fd], F32, tag="xt")
                    nc.sync.dma_start(out=t[:], in_=x_d[rows, cols])
                    nc.vector.tensor_scalar_mul(t[:], t[:], scs[rg][:])
                    nc.scalar.dma_start(out=out_d[rows, cols], in_=t[:])

    nc.compile()
    return nc


_NC_CACHE: dict = {}


def _get_nc(hw: int = HW, fd: int = 8192, bufs: int = 5):
    key = (hw, fd, bufs)
    if key not in _NC_CACHE:
        _NC_CACHE[key] = build_nc(hw, fd, bufs)
    return _NC_CACHE[key]


def make_in_maps(x, semantic, W1, b1, W2, b2, hw: int = HW):
    x = np.ascontiguousarray(np.asarray(x, dtype=np.float32))
    semantic = np.asarray(semantic, dtype=np.float32)
    w1t = np.ascontiguousarray(np.asarray(W1, dtype=np.float32).T)
    w2t = np.ascontiguousarray(np.asarray(W2, dtype=np.float32).T)
    b1c = np.ascontiguousarray(np.asarray(b1, dtype=np.float32).reshape(C, 1))
    b2r = np.ascontiguousarray(np.asarray(b2, dtype=np.float32).reshape(1, C))
    nb = x.shape[0]
    return [
        {
            "x": x[b].reshape(C, hw),
            "semantic": np.ascontiguousarray(semantic[b].reshape(C, 1)),
            "w1t": w1t,
            "b1": b1c,
            "w2t": w2t,
            "b2": b2r,
        }
        for b in range(nb)
    ]


def run(x, semantic, W1, b1, W2, b2, trace: bool = False, fd: int = 8192, bufs: int = 5):
    """Run on all 8 cores; returns (out [B,C,256,256], BassKernelResults)."""
    nc = _get_nc(HW, fd, bufs)
    in_maps = make_in_maps(x, semantic, W1, b1, W2, b2)
    res = run_bass_kernel_spmd(nc, in_maps, list(range(B)), trace=trace)
    out = np.stack(
        [res.results[i]["out"].reshape(C, 256, 256) for i in range(B)], axis=0
    )
    return out, res


def kernel(x, semantic, W1, b1, W2, b2):
    out, _ = run(x, semantic, W1, b1, W2, b2)
    return out
